# revision 26
# baseline (speedup 1.0000x reference)
"""Sharded causal-attention kernel for 8 trn2 NeuronCores.

DP over batch (2) x TP over head groups (4 heads/core). Each core: qkv projection
(its heads) + RoPE + causal SDPA (scores kept transposed; softmax denominator via a
ones-column in the PV matmul) + its 256-row slice of the o_proj contraction, returning
a transposed partial [HID, S]; the host sums 4 partials per batch. bf16 matmuls,
fp32 PSUM accumulation.

v3: pair-serial attention units sized so PSUM fits psc double-buffer (4 banks) +
one pair's PV accumulator (2 banks) + 2 general banks, letting projection /
o_proj matmuls interleave INSIDE attention units to keep the PE fed while the
ACT engine streams exps (the attention phase is exp-bound). Softmax division via
a [2,128] ones-matmul broadcast of 1/l on the PE (fp32r, ~213ns) instead of
GpSimd partition broadcasts, with the division muls reading PV numerators
directly from PSUM (no numerator SBUF copies). ACT runs exp (+ small v copies)
only; PSUM->SBUF copies go to DVE; RoPE muls/adds and causal masks go to GpSimd
(SBUF-only). rb-major wqk layout so the first projection gates on 256KB.
"""

import sys

sys.path.insert(0, "/opt/trn_rl_repo")

from contextlib import ExitStack

import numpy as np
import ml_dtypes

import concourse.bass as bass
import concourse.mybir as mybir
import concourse.tile as tile
from concourse import bacc

FP = mybir.dt.float32
FR = mybir.dt.float32r
BF = mybir.dt.bfloat16
EXP = mybir.ActivationFunctionType.Exp

B, S, HID = 2, 2048, 1024
H, D = 16, 64
QC = 512
KT = 128
NQC = S // QC
NKT = S // KT
KHID = HID // 128


def build_program(debug_outputs=False):
    nc = bacc.Bacc("TRN2", target_bir_lowering=False, debug=False, num_devices=8, num_swdge_queues=4)

    hsT = nc.dram_tensor("hsT", [128, NQC * KHID * QC], BF, kind="ExternalInput").ap()
    wqkT = nc.dram_tensor("wqkT", [128, 4 * KHID * 128], BF, kind="ExternalInput").ap()
    wvT = nc.dram_tensor("wvT", [128, KHID * 256], BF, kind="ExternalInput").ap()
    woT = nc.dram_tensor("woT", [128, 2 * HID], BF, kind="ExternalInput").ap()
    cos2T = nc.dram_tensor("cos2T", [64, S], BF, kind="ExternalInput").ap()
    ssin2T = nc.dram_tensor("ssin2T", [64, S], BF, kind="ExternalInput").ap()
    maskD = nc.dram_tensor("maskD", [128, 512], BF, kind="ExternalInput").ap()
    pmat = nc.dram_tensor("pmat", [128, 128], BF, kind="ExternalInput").ap()
    outT = nc.dram_tensor("outT", [128, NQC * 8 * QC], BF, kind="ExternalOutput").ap()
    dbg = None
    if debug_outputs:
        dbg = {
            "dbg_qk": nc.dram_tensor("dbg_qk", [512, S], BF, kind="ExternalOutput").ap(),
            "dbg_v": nc.dram_tensor("dbg_v", [128, NKT * 4 * 65], BF, kind="ExternalOutput").ap(),
            "dbg_att": nc.dram_tensor("dbg_att", [256, S], BF, kind="ExternalOutput").ap(),
        }

    with tile.TileContext(nc) as tc:
        build_tile_program(tc, hsT, wqkT, wvT, woT, cos2T, ssin2T, maskD, pmat, outT, dbg)
    nc.compile()
    return nc


def build_tile_program(tc, hsT, wqkT, wvT, woT, cos2T, ssin2T, maskD, pmat, outT, dbg=None):
    nc = tc.nc
    with ExitStack() as ctx:
        const = ctx.enter_context(tc.tile_pool(name="const", bufs=1))
        persist = ctx.enter_context(tc.tile_pool(name="persist", bufs=1))
        xp = ctx.enter_context(tc.tile_pool(name="xp", bufs=3))
        tp = ctx.enter_context(tc.tile_pool(name="tp", bufs=4))
        ep = ctx.enter_context(tc.tile_pool(name="ep", bufs=6))
        owp = ctx.enter_context(tc.tile_pool(name="owp", bufs=3))
        rlp = ctx.enter_context(tc.tile_pool(name="rlp", bufs=8))
        rbp = ctx.enter_context(tc.tile_pool(name="rbp", bufs=4))
        ps_a = ctx.enter_context(tc.tile_pool(name="ps_a", bufs=2, space="PSUM"))
        ps_sc = ctx.enter_context(tc.tile_pool(name="ps_sc", bufs=2, space="PSUM"))
        ps_po = ctx.enter_context(tc.tile_pool(name="ps_po", bufs=1, space="PSUM"))

        # ---- SBUF constants / persistent tensors ----
        wqk_sb = const.tile([128, 4, KHID, 128], BF, name="wqk_sb")
        hs_sb = const.tile([128, NQC, KHID, QC], BF, name="hs_sb")
        cos_sb = const.tile([128, S], BF, name="cos_sb")
        ssin_sb = const.tile([128, S], BF, name="ssin_sb")
        pmat_sb = const.tile([128, 128], BF, name="pmat_sb")
        tri_sb = const.tile([128, 4, 128], BF, name="tri_sb")
        wv_sb = const.tile([128, KHID, 256], BF, name="wv_sb")
        wo_sb = const.tile([128, 2, HID], BF, name="wo_sb")

        qkT = persist.tile([128, 4, S], BF, name="qkT")
        v_sb = persist.tile([128, NKT, 4 * 65], BF, name="v_sb2")
        att_sb = persist.tile([128, 2, S], BF, name="att_sb2")

        # ---- wave-1 DMAs: exactly what P(0)+U(0) gate on, in consumption
        # order, spread across queues (sequencer descriptor writes are ~0.6us
        # each, so keep them off hot engines: scalar gets only small early
        # ones, the rest ride sync/tensor/gpsimd/vector which are idle while
        # the data they deliver is still in flight) ----
        def wq_rb(rb, eng):
            eng.dma_start(
                wqk_sb[:, rb],
                wqkT[:, rb * 1024:(rb + 1) * 1024].rearrange("p (k m) -> p k m", k=KHID),
            )

        def hs_piece(t, klo, khi, eng):
            eng.dma_start(
                hs_sb[:, t, klo:khi, :],
                hsT[:, (t * KHID + klo) * QC:(t * KHID + khi) * QC].rearrange(
                    "p (k s) -> p k s", k=khi - klo),
            )

        nc.sync.dma_start(wqk_sb[:, 0], wqkT[:, 0:1024].rearrange("p (k m) -> p k m", k=KHID))
        hs_piece(0, 0, 2, nc.gpsimd)
        nc.scalar.dma_start(cos_sb[0:64, 0:QC], cos2T[:, 0:QC])
        hs_piece(0, 2, 4, nc.gpsimd)
        nc.scalar.dma_start(ssin_sb[0:64, 0:QC], ssin2T[:, 0:QC])
        nc.scalar.dma_start(pmat_sb[:], pmat[:])
        wq_rb(2, nc.scalar)
        hs_piece(0, 4, 6, nc.sync)
        wq_rb(1, nc.sync)
        hs_piece(0, 6, 8, nc.gpsimd)
        wq_rb(3, nc.sync)
        for h in range(2):
            nc.sync.dma_start(
                wv_sb[:, 4 * h:4 * h + 4, :],
                wvT[:, h * 1024:(h + 1) * 1024].rearrange("p (k m) -> p k m", k=4),
            )
        nc.scalar.dma_start(tri_sb[:], maskD.rearrange("p (r c) -> p r c", r=4))
        # wave 2: big deferred loads, one descriptor each
        hs_piece(1, 0, 8, nc.gpsimd)
        hs_piece(2, 0, 8, nc.gpsimd)
        hs_piece(3, 0, 8, nc.gpsimd)
        for h in range(2):
            nc.scalar.dma_start(wo_sb[:, h, :], woT[:, h * HID:(h + 1) * HID])
        nc.sync.dma_start(cos_sb[0:64, QC:S], cos2T[:, QC:S])
        nc.sync.dma_start(ssin_sb[0:64, QC:S], ssin2T[:, QC:S])

        # small SBUF init work while DMAs fly
        nc.vector.tensor_copy(cos_sb[64:128, 0:QC], cos_sb[0:64, 0:QC])
        nc.vector.tensor_copy(ssin_sb[64:128, 0:QC], ssin_sb[0:64, 0:QC])
        nc.vector.memset(
            v_sb.rearrange("p t (h c) -> p t h c", c=65)[:, :, :, 64:65], 1.0
        )

        late_cs = [False]

        def late_cos_sin():
            if not late_cs[0]:
                late_cs[0] = True
                nc.vector.tensor_copy(cos_sb[64:128, QC:S], cos_sb[0:64, QC:S])
                nc.vector.tensor_copy(ssin_sb[64:128, QC:S], ssin_sb[0:64, QC:S])

        # ---- building blocks ----
        def proj_rb(rb, t):
            """q/k projection of row-block rb for chunk t, with RoPE into qkT."""
            csl = slice(t * QC, (t + 1) * QC)
            ps = ps_a.tile([128, QC], FP, name="ps", tag="a")
            for kk in range(KHID):
                nc.tensor.matmul(
                    ps[:], wqk_sb[:, rb, kk, :], hs_sb[:, t, kk, :],
                    start=(kk == 0), stop=(kk == KHID - 1),
                )
            x = xp.tile([128, QC], BF, name="x", tag="x")
            nc.vector.tensor_copy(x[:], ps[:])
            xs = ps_a.tile([128, QC], FP, name="xs", tag="a")
            nc.tensor.matmul(xs[:], pmat_sb[:], x[:], start=True, stop=True)
            t1 = tp.tile([128, QC], BF, name="t1", tag="t")
            t2 = tp.tile([128, QC], BF, name="t2", tag="t")
            nc.gpsimd.tensor_mul(t1[:], x[:], cos_sb[:, csl])
            nc.vector.tensor_mul(t2[:], xs[:], ssin_sb[:, csl])
            nc.gpsimd.tensor_add(qkT[:, rb, csl], t1[:], t2[:])

        def v_proj(tt):
            psv = ps_a.tile([128, 256], FP, name="psv", tag="a")
            for kk in range(KHID):
                nc.tensor.matmul(
                    psv[:],
                    hs_sb[:, tt // 4, kk, (tt % 4) * 128:(tt % 4 + 1) * 128],
                    wv_sb[:, kk, :],
                    start=(kk == 0), stop=(kk == KHID - 1),
                )
            nc.scalar.copy(
                v_sb[:, tt, :].rearrange("p (h c) -> p h c", c=65)[:, :, 0:64],
                psv[:].rearrange("p (h c) -> p h c", c=64),
            )

        def pair_unit(qi, p):
            """Generator: scores->exp->PV for heads (2p, 2p+1) at q-chunk qi.

            Yields after each k-tile so callers can interleave PE filler work.
            PV trails one k-tile behind exp. Epilogue: 1/l via fast recip,
            [2,128] ones-matmul broadcast on the PE (fp32r), division muls
            reading the PV numerators straight from PSUM into att_sb."""
            qsl = slice(qi * QC, (qi + 1) * QC)
            nki = 4 * qi + 4
            po = ps_po.tile([65, 2, QC], FP, name="po", tag="po")
            pending = None

            def emit_pv(ki, e_, lo):
                for h in range(2):
                    nc.tensor.matmul(
                        po[:, h, lo:QC],
                        v_sb[:, ki, (2 * p + h) * 65:(2 * p + h + 1) * 65],
                        e_[:, h, lo:QC],
                        start=(ki == 0), stop=(ki == nki - 1),
                    )

            for ki in range(nki):
                ksl = slice(ki * KT, (ki + 1) * KT)
                j = ki - 4 * qi
                lo = 0 if j < 0 else 128 * j
                psc = ps_sc.tile([128, 2, QC], FP, name="psc", tag="sc")
                nc.tensor.matmul(
                    psc[:, 0, lo:QC], qkT[0:64, 2 + p, ksl],
                    qkT[0:64, p, qi * QC + lo:(qi + 1) * QC],
                    start=True, stop=True,
                )
                nc.tensor.matmul(
                    psc[:, 1, lo:QC], qkT[64:128, 2 + p, ksl],
                    qkT[64:128, p, qi * QC + lo:(qi + 1) * QC],
                    start=True, stop=True,
                )
                e = ep.tile([128, 2, QC], BF, name="e", tag="e")
                nc.scalar.activation(e[:, :, lo:QC], psc[:, :, lo:QC], EXP, scale=0.125)
                if j >= 0:
                    nc.gpsimd.tensor_mul(
                        e[:, :, lo:lo + 128], e[:, :, lo:lo + 128], tri_sb[:, 0:2, :]
                    )
                if pending is not None:
                    emit_pv(*pending)
                pending = (ki, e, lo)
                yield
            emit_pv(*pending)
            # epilogue: per-head reciprocal of the ones-column sums, PE
            # broadcast to 128 partitions, divide numerators from PSUM.
            # reciprocal_approx_fast reads garbage from PSUM on HW (sim
            # divergence) - stage the ones-column sums through SBUF first
            lA = rlp.tile([1, QC], FP, name="lA", tag="rl")
            lB = rlp.tile([1, QC], FP, name="lB", tag="rl")
            nc.vector.tensor_copy(lA[:], po[64:65, 0, :])
            nc.vector.tensor_copy(lB[:], po[64:65, 1, :])
            rlA = rlp.tile([1, QC], FP, name="rlA", tag="rl")
            rlB = rlp.tile([1, QC], FP, name="rlB", tag="rl")
            nc.vector.reciprocal_approx_fast(out=rlA[:], in_=lA[:])
            nc.vector.reciprocal_approx_fast(out=rlB[:], in_=lB[:])
            rbA = rbp.tile([64, QC], FP, name="rbA", tag="rbs")
            rbB = rbp.tile([64, QC], FP, name="rbB", tag="rbs")
            nc.gpsimd.partition_broadcast(rbA[:], rlA[:])
            nc.gpsimd.partition_broadcast(rbB[:], rlB[:])
            nc.vector.tensor_mul(att_sb[0:64, p, qsl], po[0:64, 0, :], rbA[:])
            nc.vector.tensor_mul(att_sb[64:128, p, qsl], po[0:64, 1, :], rbB[:])
            yield

        OUT_ENGS = (nc.sync, nc.gpsimd, nc.sync, nc.gpsimd, nc.sync, nc.gpsimd, nc.sync, nc.gpsimd)

        def oproj_half(qi, half):
            qsl = slice(qi * QC, (qi + 1) * QC)
            ow = owp.tile([128, 4, QC], BF, name="ow", tag="ow")
            for oi in range(4):
                ot = half * 4 + oi
                pw = ps_a.tile([128, QC], FP, name="pw", tag="a")
                for p in range(2):
                    nc.tensor.matmul(
                        pw[:], wo_sb[:, p, ot * 128:(ot + 1) * 128], att_sb[:, p, qsl],
                        start=(p == 0), stop=(p == 1),
                    )
                if oi % 2 == 0:
                    nc.vector.tensor_copy(ow[:, oi, :], pw[:])
                else:
                    nc.scalar.copy(ow[:, oi, :], pw[:])
            off = (qi * 2 + half) * 4 * QC
            OUT_ENGS[qi * 2 + half].dma_start(
                outT[:, off:off + 4 * QC].rearrange("p (o s) -> p o s", o=4), ow[:]
            )

        # ---- emission: P(0) full, then pair-units with proj/oproj fillers
        # interleaved at k-tile boundaries ----
        for rb_ in (0, 2, 1, 3):
            proj_rb(rb_, 0)
        for tt in range(4):
            v_proj(tt)

        def fillers_for(qi, p):
            """PE filler tasks to interleave into unit (qi, p), keyed by the
            k-tile index BEFORE which they must be emitted (dependencies) or
            just spread for pacing. Returns dict ki -> list of callables."""
            f = {}
            if qi == 0 and p == 0:
                f[2] = [late_cos_sin, lambda: proj_rb(0, 1)]
                f[3] = [lambda: proj_rb(2, 1)]
            if qi == 0 and p == 1:
                f[1] = [lambda: proj_rb(1, 1)]
                f[2] = [lambda: proj_rb(3, 1)]
                f[3] = [lambda: v_proj(4), lambda: v_proj(5)]
            if qi == 1 and p == 0:
                f[1] = [lambda: v_proj(6), lambda: v_proj(7)]
                f[3] = [lambda: proj_rb(0, 2)]
                f[5] = [lambda: proj_rb(2, 2)]
            if qi == 1 and p == 1:
                f[1] = [lambda: oproj_half(0, 0)]
                f[3] = [lambda: oproj_half(0, 1)]
                f[5] = [lambda: proj_rb(1, 2), lambda: proj_rb(3, 2)]
            if qi == 2 and p == 0:
                f[1] = [lambda: v_proj(8), lambda: v_proj(9)]
                f[3] = [lambda: v_proj(10), lambda: v_proj(11)]
                f[5] = [lambda: oproj_half(1, 0)]
                f[7] = [lambda: oproj_half(1, 1)]
                f[9] = [lambda: proj_rb(0, 3)]
            if qi == 2 and p == 1:
                f[1] = [lambda: proj_rb(2, 3)]
                f[4] = [lambda: proj_rb(1, 3)]
                f[7] = [lambda: proj_rb(3, 3)]
                f[10] = [lambda: v_proj(12)]
            if qi == 3 and p == 0:
                f[1] = [lambda: v_proj(13), lambda: v_proj(14)]
                f[3] = [lambda: v_proj(15)]
                f[6] = [lambda: oproj_half(2, 0)]
                f[10] = [lambda: oproj_half(2, 1)]
            return f

        for qi in range(NQC):
            for p in range(2):
                f = fillers_for(qi, p)
                for ki, _ in enumerate(pair_unit(qi, p)):
                    for task in f.get(ki, ()):
                        task()
        oproj_half(3, 0)
        oproj_half(3, 1)

        if dbg is not None:
            for rb_ in range(4):
                nc.sync.dma_start(dbg["dbg_qk"][rb_ * 128:(rb_ + 1) * 128, :], qkT[:, rb_, :])
            nc.sync.dma_start(dbg["dbg_v"][:], v_sb.rearrange("p t c -> p (t c)"))
            for p in range(2):
                nc.sync.dma_start(dbg["dbg_att"][p * 128:(p + 1) * 128, :], att_sb[:, p, :])


# ---------- host-side shard preparation ----------

def make_core_inputs(hidden_states, cos, sin, w_qkv, w_o):
    """Returns list of 8 in_maps (numpy, bf16 where needed)."""
    bf = ml_dtypes.bfloat16
    hs = np.asarray(hidden_states, np.float32)
    cos = np.asarray(cos, np.float32)
    sin = np.asarray(sin, np.float32)
    w_qkv = np.asarray(w_qkv, np.float32)
    w_o = np.asarray(w_o, np.float32)

    cos2T = np.ascontiguousarray(cos.T).astype(bf)
    ssin2T = np.ascontiguousarray(sin.T).astype(bf)
    # signed rotate-half permutation: out[m] = sign(m) * x[partner(m)]
    # lhsT layout: pmat[k, m] = sign(m) at k = partner(m)
    pmat = np.zeros((128, 128), np.float32)
    for m in range(128):
        d = m % 64
        base = m - d
        if d < 32:
            pmat[base + d + 32, m] = -1.0
        else:
            pmat[base + d - 32, m] = 1.0
    pmat = pmat.astype(bf)

    kp = np.arange(128)[:, None]
    cc = np.arange(128)[None, :]
    tri = (kp <= cc).astype(bf)
    maskD = np.concatenate([tri, tri, tri, tri], axis=1)

    def swz(a):
        # [K*128, M] -> [128, K*M] partition-major (matches SBUF tiles)
        k = a.shape[0] // 128
        return np.ascontiguousarray(
            a.reshape(k, 128, a.shape[1]).transpose(1, 0, 2).reshape(128, -1))

    in_maps = []
    for c in range(8):
        b, g = divmod(c, 4)
        heads = range(4 * g, 4 * g + 4)
        hsT = np.ascontiguousarray(hs[b].T).astype(bf)  # [HID, S]
        # chunk-major swizzle: [128, t, kk, s']
        hs2 = np.ascontiguousarray(
            hsT.reshape(8, 128, 4, 512).transpose(1, 2, 0, 3).reshape(128, -1))
        wq = np.concatenate([w_qkv[h * 64:(h + 1) * 64] for h in heads], 0)
        wk = np.concatenate([w_qkv[HID + h * 64:HID + (h + 1) * 64] for h in heads], 0)
        wv = np.concatenate([w_qkv[2 * HID + h * 64:2 * HID + (h + 1) * 64] for h in heads], 0)
        # rb-major wqk: [128, rb, kk, m]
        wqk_cat = np.concatenate([wq, wk], 0).astype(np.float32)  # [512, 1024]
        arr = np.ascontiguousarray(wqk_cat.T)  # [1024, 512] = [kk*128+p, rb*128+m]
        wqkT = np.ascontiguousarray(
            arr.reshape(KHID, 128, 4, 128).transpose(1, 2, 0, 3).reshape(128, -1)
        ).astype(bf)
        wvT = swz(np.ascontiguousarray(wv.T).astype(bf))
        woT = swz(np.ascontiguousarray(
            np.concatenate([w_o[:, h * 64:(h + 1) * 64] for h in heads], 1).T
        ).astype(bf))
        in_maps.append({
            "hsT": hs2, "wqkT": wqkT, "wvT": wvT, "woT": woT,
            "cos2T": cos2T, "ssin2T": ssin2T, "maskD": maskD, "pmat": pmat,
        })
    return in_maps


def unswizzle_out(o2):
    # [128, qi*half*oi*512] -> outT [1024, 2048]
    a = o2.reshape(128, NQC, 2, 4, QC)
    return np.ascontiguousarray(
        a.transpose(2, 3, 0, 1, 4).reshape(HID, S))


def unshard(outTs):
    out = np.zeros((B, S, HID), np.float32)
    for c, o2 in enumerate(outTs):
        out[c // 4] += unswizzle_out(o2).T.astype(np.float32)
    return out


# ---------- standalone kernel entry ----------

from concourse.bass_utils import run_bass_kernel_spmd

_CACHED_NC = None


def get_program():
    global _CACHED_NC
    if _CACHED_NC is None:
        _CACHED_NC = build_program()
    return _CACHED_NC


def run(inputs, trace=False):
    nc = get_program()
    in_maps = make_core_inputs(**inputs)
    res = run_bass_kernel_spmd(nc, in_maps, core_ids=list(range(8)), trace=trace)
    out = np.zeros((B, S, HID), np.float32)
    for c, r in enumerate(res.results):
        out[c // 4] += unswizzle_out(r["outT"]).T.astype(np.float32)
    return out, res


def kernel(**inputs):
    out, _ = run(inputs, trace=False)
    return out


# revision 38
# speedup vs baseline: 1.3138x; 1.3138x over previous
"""Sharded causal-attention kernel for 8 trn2 NeuronCores.

DP over batch (2) x TP over head groups (4 heads/core). Each core: qkv projection
(its heads) + RoPE + causal SDPA (scores kept transposed; softmax denominator via a
ones-column in the PV matmul) + its 256-row slice of the o_proj contraction, returning
a transposed partial [HID, S]; the host sums 4 partials per batch. bf16 matmuls,
fp32 PSUM accumulation.

v4 = v2 schedule skeleton with the ACT engine reserved for exp (its 72us pole):
every PSUM->SBUF copy (RoPE x, v, numerators, denominator rows, o_proj out)
moved to DVE with denominator rows copied first so the recip/broadcast chain
starts earliest; RoPE t1 mul and final add moved to GpSimd (SBUF-only ops,
~780ns each, keeps DVE under the PE pole); rb-major wqk layout so the first
projection row-block gates on one 256KB transfer; hs chunks 1-3 as single
8-piece transfers on the gpsimd queue.
"""

import sys

sys.path.insert(0, "/opt/trn_rl_repo")

from contextlib import ExitStack

import numpy as np
import ml_dtypes

import concourse.bass as bass
import concourse.mybir as mybir
import concourse.tile as tile
from concourse import bacc

FP = mybir.dt.float32
BF = mybir.dt.bfloat16
EXP = mybir.ActivationFunctionType.Exp

B, S, HID = 2, 2048, 1024
H, D = 16, 64
QC = 512
KT = 128
NQC = S // QC
NKT = S // KT
KHID = HID // 128


def build_program(debug_outputs=False):
    nc = bacc.Bacc("TRN2", target_bir_lowering=False, debug=False, num_devices=8, num_swdge_queues=4)

    hsT = nc.dram_tensor("hsT", [128, NQC * KHID * QC], BF, kind="ExternalInput").ap()
    wqkT = nc.dram_tensor("wqkT", [128, KHID * 512], BF, kind="ExternalInput").ap()
    wvT = nc.dram_tensor("wvT", [128, KHID * 256], BF, kind="ExternalInput").ap()
    woT = nc.dram_tensor("woT", [128, 2 * HID], BF, kind="ExternalInput").ap()
    cos2T = nc.dram_tensor("cos2T", [64, S], BF, kind="ExternalInput").ap()
    ssin2T = nc.dram_tensor("ssin2T", [64, S], BF, kind="ExternalInput").ap()
    maskD = nc.dram_tensor("maskD", [128, 512], BF, kind="ExternalInput").ap()
    pmat = nc.dram_tensor("pmat", [128, 128], BF, kind="ExternalInput").ap()
    outT = nc.dram_tensor("outT", [128, NQC * 8 * QC], BF, kind="ExternalOutput").ap()
    dbg = None
    if debug_outputs:
        dbg = {
            "dbg_qk": nc.dram_tensor("dbg_qk", [512, S], BF, kind="ExternalOutput").ap(),
            "dbg_v": nc.dram_tensor("dbg_v", [128, NKT * 4 * 65], BF, kind="ExternalOutput").ap(),
            "dbg_att": nc.dram_tensor("dbg_att", [256, S], BF, kind="ExternalOutput").ap(),
        }

    with tile.TileContext(nc) as tc:
        build_tile_program(tc, hsT, wqkT, wvT, woT, cos2T, ssin2T, maskD, pmat, outT, dbg)
    nc.compile()
    return nc


def build_tile_program(tc, hsT, wqkT, wvT, woT, cos2T, ssin2T, maskD, pmat, outT, dbg=None):
    nc = tc.nc
    with ExitStack() as ctx:
        const = ctx.enter_context(tc.tile_pool(name="const", bufs=1))
        persist = ctx.enter_context(tc.tile_pool(name="persist", bufs=1))
        work = ctx.enter_context(tc.tile_pool(name="work", bufs=5))
        posbp = ctx.enter_context(tc.tile_pool(name="posbp", bufs=12))
        expp = ctx.enter_context(tc.tile_pool(name="expp", bufs=8))
        small = ctx.enter_context(tc.tile_pool(name="small", bufs=4))
        ps_main = ctx.enter_context(tc.tile_pool(name="ps_main", bufs=4, space="PSUM"))
        ps_sc = ctx.enter_context(tc.tile_pool(name="ps_sc", bufs=2, space="PSUM"))

        # ---- critical first wave: exactly what the first proj matmuls gate on,
        # in consumption order (rb-major wqk so proj rb0 gates on 256KB),
        # spread across the three DMA queues ----
        wqk_sb = const.tile([128, 4, KHID, 128], BF, name="wqk_sb")
        hs_sb = const.tile([128, NQC, KHID, QC], BF, name="hs_sb")
        cos_sb = const.tile([128, S], BF, name="cos_sb")
        ssin_sb = const.tile([128, S], BF, name="ssin_sb")
        pmat_sb = const.tile([128, 128], BF, name="pmat_sb")
        tri_sb = const.tile([128, 4, 128], BF, name="tri_sb")
        wv_sb = const.tile([128, KHID, 256], BF, name="wv_sb")
        wo_sb = const.tile([128, 2, HID], BF, name="wo_sb")

        def wq_rb(rb, eng):
            eng.dma_start(
                wqk_sb[:, rb],
                wqkT[:, rb * 1024:(rb + 1) * 1024].rearrange("p (k m) -> p k m", k=KHID),
            )

        def hs_piece(t, klo, khi, eng):
            eng.dma_start(
                hs_sb[:, t, klo:khi, :],
                hsT[:, (t * KHID + klo) * QC:(t * KHID + khi) * QC].rearrange(
                    "p (k s) -> p k s", k=khi - klo),
            )

        wq_rb(0, nc.sync)
        hs_piece(0, 0, 2, nc.gpsimd)
        nc.scalar.dma_start(cos_sb[0:64, 0:QC], cos2T[:, 0:QC])
        hs_piece(0, 2, 4, nc.gpsimd)
        nc.scalar.dma_start(ssin_sb[0:64, 0:QC], ssin2T[:, 0:QC])
        nc.scalar.dma_start(pmat_sb[:], pmat[:])
        wq_rb(2, nc.scalar)
        hs_piece(0, 4, 6, nc.sync)
        wq_rb(1, nc.sync)
        hs_piece(0, 6, 8, nc.gpsimd)
        wq_rb(3, nc.sync)
        # remaining hs chunks ride the gpsimd queue behind the chunk-0 pieces
        hs_piece(1, 0, 8, nc.gpsimd)
        hs_piece(2, 0, 8, nc.gpsimd)
        hs_piece(3, 0, 8, nc.gpsimd)
        nc.vector.tensor_copy(cos_sb[64:128, 0:QC], cos_sb[0:64, 0:QC])
        nc.vector.tensor_copy(ssin_sb[64:128, 0:QC], ssin_sb[0:64, 0:QC])

        qkT = persist.tile([128, 4, S], BF, name="qkT")
        l_tiles = persist.tile([64, 8, QC], FP, name="l_tiles")
        v_sb = persist.tile([128, NKT, 4 * 65], BF, name="v_sb2")
        att_sb = persist.tile([128, 2, S], BF, name="att_sb2")
        nc.vector.memset(
            v_sb.rearrange("p t (h c) -> p t h c", c=65)[:, :, :, 64:65], 1.0
        )

        def proj_chunk(rb, t):
            csl = slice(t * QC, (t + 1) * QC)
            ps = ps_main.tile([128, QC], FP, name="ps_qk", tag="ps")
            for kk in range(KHID):
                nc.tensor.matmul(
                    ps[:],
                    wqk_sb[:, rb, kk, :],
                    hs_sb[:, t, kk, :],
                    start=(kk == 0),
                    stop=(kk == KHID - 1),
                )
            x = work.tile([128, QC], BF, name="x_rope", tag="xrope")
            half = QC // 2
            nc.vector.tensor_copy(x[:], ps[:])
            # signed rotate-half on the PE: xs = P @ x (P carries the +-1)
            xs_ps = ps_main.tile([128, QC], FP, name="xs_ps", tag="ps")
            nc.tensor.matmul(xs_ps[:], pmat_sb[:], x[:], start=True, stop=True)
            t1 = work.tile([128, QC], BF, name="t1_rope", tag="t1rope")
            t2 = work.tile([128, QC], BF, name="t2_rope", tag="t2rope")
            nc.gpsimd.tensor_mul(t1[:], x[:], cos_sb[:, csl])
            nc.vector.tensor_mul(t2[:], xs_ps[:], ssin_sb[:, csl])
            nc.gpsimd.tensor_add(qkT[:, rb, csl], t1[:], t2[:])

        def v_proj(tt):
            psv = ps_main.tile([128, 256], FP, name="ps_v", tag="ps")
            for kk in range(KHID):
                nc.tensor.matmul(
                    psv[:],
                    hs_sb[:, tt // 4, kk, (tt % 4) * 128:(tt % 4 + 1) * 128],
                    wv_sb[:, kk, :],
                    start=(kk == 0),
                    stop=(kk == KHID - 1),
                )
            nc.vector.tensor_copy(
                v_sb[:, tt, :].rearrange("p (h c) -> p h c", c=65)[:, :, 0:64],
                psv[:].rearrange("p (h c) -> p h c", c=64),
            )

        def attention_unit(qi, l0, l1):
            """Fused scores^T -> exp -> PV for all four heads at q-chunk qi.

            Both head-pairs interleave per k-tile so each exp has ~2x the PE
            cover, and the PV batch trails one k-tile behind. The four PV
            accumulators occupy all four ps_main buffers for the unit.
            Denominators land in l0/l1 rows 0 and 32."""
            qsl = slice(qi * QC, (qi + 1) * QC)
            nki = 4 * qi + 4
            po = [ps_main.tile([65, QC], FP, name=f"po{h}", tag="ps") for h in range(4)]

            def emit_pv(ki, e_, lo):
                for h in range(4):
                    nc.tensor.matmul(
                        po[h][:, lo:QC], v_sb[:, ki, h * 65:(h + 1) * 65],
                        e_[:, h, lo:QC],
                        start=(ki == 0), stop=(ki == nki - 1),
                    )

            pending = []
            for ki in range(nki):
                ksl = slice(ki * KT, (ki + 1) * KT)
                j = ki - 4 * qi
                lo = 0 if j < 0 else 128 * j  # first live q column in this chunk
                e = expp.tile([128, 4, QC], BF, name="e", tag="exp", bufs=4)
                for pair in range(2):
                    psc = ps_sc.tile([128, 2, QC], FP, name="psc", tag="sc")
                    nc.tensor.matmul(
                        psc[:, 0, lo:QC], qkT[0:64, 2 + pair, ksl],
                        qkT[0:64, pair, qi * QC + lo:(qi + 1) * QC],
                        start=True, stop=True,
                    )
                    nc.tensor.matmul(
                        psc[:, 1, lo:QC], qkT[64:128, 2 + pair, ksl],
                        qkT[64:128, pair, qi * QC + lo:(qi + 1) * QC],
                        start=True, stop=True,
                    )
                    nc.scalar.activation(
                        e[:, 2 * pair:2 * pair + 2, lo:QC], psc[:, :, lo:QC],
                        EXP, scale=0.125,
                    )
                if j >= 0:
                    nc.vector.tensor_mul(
                        e[:, :, lo:lo + 128], e[:, :, lo:lo + 128], tri_sb[:]
                    )
                if pending:
                    emit_pv(*pending.pop(0))
                pending.append((ki, e, lo))
            for p_ in pending:
                emit_pv(*p_)
            # epilogue (all DVE so exps never queue behind copies):
            # denominator rows first so the recip/broadcast chain starts
            # earliest, then the numerator copies
            nc.vector.tensor_copy(l0[0:1, :], po[0][64:65, :])
            nc.vector.tensor_copy(l0[32:33, :], po[1][64:65, :])
            nc.vector.tensor_copy(l1[0:1, :], po[2][64:65, :])
            nc.vector.tensor_copy(l1[32:33, :], po[3][64:65, :])
            po_sb = []
            for h in range(4):
                t_ = posbp.tile([64, QC], BF, name=f"po_sb{h}", tag="posb")
                nc.vector.tensor_copy(t_[:], po[h][0:64, :])
                po_sb.append(t_)
            return (po_sb[0], po_sb[1]), (po_sb[2], po_sb[3])

        def division_pre(l_pair):
            """1/l for both heads of a pair, broadcast to 64 partitions.

            Only depends on the denominator rows, so the gpsimd broadcasts
            overlap the next attention unit / o_proj on the PE."""
            rl = small.tile([64, QC], FP, name="rl", tag="rl")
            nc.vector.reciprocal_approx_fast(out=rl[:], in_=l_pair[:])
            rb0_ = small.tile([64, QC], FP, name="rb0_", tag="rbb", bufs=6)
            nc.gpsimd.partition_broadcast(rb0_[:], rl[0:1, :])
            rlrow = small.tile([1, QC], FP, name="rlrow", tag="rlrow", bufs=4)
            nc.vector.tensor_copy(rlrow[:], rl[32:33, :])
            rb1_ = small.tile([64, QC], FP, name="rb1_", tag="rbb", bufs=6)
            nc.gpsimd.partition_broadcast(rb1_[:], rlrow[:])
            return rb0_, rb1_

        def division_post(pair, qi, rb, po_sb0, po_sb1):
            qsl = slice(qi * QC, (qi + 1) * QC)
            rb0_, rb1_ = rb
            nc.vector.tensor_mul(att_sb[0:64, pair, qsl], po_sb0[0:64, :], rb0_[:])
            nc.vector.tensor_mul(att_sb[64:128, pair, qsl], po_sb1[0:64, :], rb1_[:])

        def oproj(qi, last=False, tailish=False):
            qsl = slice(qi * QC, (qi + 1) * QC)
            out_engs = (nc.gpsimd, nc.sync, nc.scalar, nc.gpsimd)
            if last:
                for half in range(2):
                    ow = work.tile([128, 4, QC], BF, name="ow", tag="ow")
                    for oi in range(4):
                        ot = half * 4 + oi
                        pw = ps_main.tile([128, QC], FP, name="pw", tag="ps")
                        for p in range(2):
                            nc.tensor.matmul(
                                pw[:],
                                wo_sb[:, p, ot * 128:(ot + 1) * 128],
                                att_sb[:, p, qsl],
                                start=(p == 0),
                                stop=(p == 1),
                            )
                        nc.vector.tensor_copy(ow[:, oi, :], pw[:])
                        off = (qi * 8 + half * 4 + oi) * QC
                        out_engs[oi].dma_start(outT[:, off:off + QC], ow[:, oi, :])
                return
            for half in range(2):
                ow = work.tile([128, 4, QC], BF, name="ow", tag="ow")
                for oi in range(4):
                    ot = half * 4 + oi
                    pw = ps_main.tile([128, QC], FP, name="pw", tag="ps")
                    for p in range(2):
                        nc.tensor.matmul(
                            pw[:],
                            wo_sb[:, p, ot * 128:(ot + 1) * 128],
                            att_sb[:, p, qsl],
                            start=(p == 0),
                            stop=(p == 1),
                        )
                    nc.vector.tensor_copy(ow[:, oi, :], pw[:])
                off = (qi * 2 + half) * 4 * QC
                (nc.gpsimd if half == 0 else nc.sync).dma_start(
                    outT[:, off:off + 4 * QC].rearrange("p (o s) -> p o s", o=4),
                    ow[:],
                )

        # emission: pair0 projections up front with the deferred weight loads
        # slotted behind compute-gated ops so they don't steal SDMA bandwidth
        # from the chunk-0 / wqk gates; then pair1 projections, v, attention
        # and (one chunk behind) o_proj interleaved per q chunk.
        for t in range(NQC):
            if t == 1:
                nc.vector.tensor_copy(cos_sb[64:128, QC:S], cos_sb[0:64, QC:S])
                nc.vector.tensor_copy(ssin_sb[64:128, QC:S], ssin_sb[0:64, QC:S])
            proj_chunk(0, t)
            if t == 0:
                nc.scalar.dma_start(cos_sb[0:64, QC:S], cos2T[:, QC:S])
                nc.scalar.dma_start(ssin_sb[0:64, QC:S], ssin2T[:, QC:S])
                nc.scalar.dma_start(tri_sb[:], maskD.rearrange("p (r c) -> p r c", r=4))
            proj_chunk(2, t)
            if t == 0:
                for h in range(2):
                    nc.scalar.dma_start(
                        wv_sb[:, 4 * h:4 * h + 4, :],
                        wvT[:, h * 1024:(h + 1) * 1024].rearrange("p (k m) -> p k m", k=4),
                    )
            if t == 1:
                for h in range(2):
                    nc.scalar.dma_start(wo_sb[:, h, :], woT[:, h * HID:(h + 1) * HID])
        nc.vector.memset(l_tiles[:], 1.0)
        qi_order = [1, 0, 2, 3]
        loaded = 0
        prev = None
        for qi in qi_order:
            while loaded <= min(qi + 1, NQC - 1):
                proj_chunk(1, loaded)
                proj_chunk(3, loaded)
                for tt in range(4 * loaded, 4 * loaded + 4):
                    v_proj(tt)
                loaded += 1
            l0 = l_tiles[:, 2 * qi, :]
            l1 = l_tiles[:, 2 * qi + 1, :]
            pa, pb = attention_unit(qi, l0, l1)
            rlb0 = division_pre(l0)
            rlb1 = division_pre(l1)
            if prev is not None:
                oproj(prev, tailish=(qi == qi_order[-1]))
            division_post(0, qi, rlb0, *pa)
            division_post(1, qi, rlb1, *pb)
            prev = qi
        oproj(prev, last=True)

        if dbg is not None:
            for rb in range(4):
                nc.sync.dma_start(dbg["dbg_qk"][rb * 128:(rb + 1) * 128, :], qkT[:, rb, :])
            nc.sync.dma_start(dbg["dbg_v"][:], v_sb.rearrange("p t c -> p (t c)"))
            for p in range(2):
                nc.sync.dma_start(dbg["dbg_att"][p * 128:(p + 1) * 128, :], att_sb[:, p, :])


# ---------- host-side shard preparation ----------

def make_core_inputs(hidden_states, cos, sin, w_qkv, w_o):
    """Returns list of 8 in_maps (numpy, bf16 where needed)."""
    bf = ml_dtypes.bfloat16
    hs = np.asarray(hidden_states, np.float32)
    cos = np.asarray(cos, np.float32)
    sin = np.asarray(sin, np.float32)
    w_qkv = np.asarray(w_qkv, np.float32)
    w_o = np.asarray(w_o, np.float32)

    cosT = cos.T
    sinT = sin.T
    cos2T = np.ascontiguousarray(cosT).astype(bf)
    ssin2T = np.ascontiguousarray(sinT).astype(bf)
    # signed rotate-half permutation: out[m] = sign(m) * x[partner(m)]
    # lhsT layout: pmat[k, m] = sign(m) at k = partner(m)
    pmat = np.zeros((128, 128), np.float32)
    for m in range(128):
        d = m % 64
        base = m - d
        if d < 32:
            pmat[base + d + 32, m] = -1.0
        else:
            pmat[base + d - 32, m] = 1.0
    pmat = pmat.astype(bf)

    kp = np.arange(128)[:, None]
    cc = np.arange(128)[None, :]
    tri = (kp <= cc).astype(bf)
    maskD = np.concatenate([tri, tri, tri, tri], axis=1)

    def swz(a):
        # [K*128, M] -> [128, K*M] partition-major (matches SBUF tiles)
        k = a.shape[0] // 128
        return np.ascontiguousarray(
            a.reshape(k, 128, a.shape[1]).transpose(1, 0, 2).reshape(128, -1))

    in_maps = []
    for c in range(8):
        b, g = divmod(c, 4)
        heads = range(4 * g, 4 * g + 4)
        hsT = np.ascontiguousarray(hs[b].T).astype(bf)  # [HID, S]
        # chunk-major swizzle: [128, t, kk, s']
        hs2 = np.ascontiguousarray(
            hsT.reshape(8, 128, 4, 512).transpose(1, 2, 0, 3).reshape(128, -1))
        wq = np.concatenate([w_qkv[h * 64:(h + 1) * 64] for h in heads], 0)
        wk = np.concatenate([w_qkv[HID + h * 64:HID + (h + 1) * 64] for h in heads], 0)
        wv = np.concatenate([w_qkv[2 * HID + h * 64:2 * HID + (h + 1) * 64] for h in heads], 0)
        # rb-major wqk: [128, rb, kk, m] so proj of one 128-row block gates
        # on a single 256KB transfer
        wqk_cat = np.concatenate([wq, wk], 0).astype(np.float32)  # [512, 1024]
        arr = np.ascontiguousarray(wqk_cat.T)  # [1024, 512] = [kk*128+p, rb*128+m]
        wqkT = np.ascontiguousarray(
            arr.reshape(KHID, 128, 4, 128).transpose(1, 2, 0, 3).reshape(128, -1)
        ).astype(bf)
        wvT = swz(np.ascontiguousarray(wv.T).astype(bf))
        woT = swz(np.ascontiguousarray(
            np.concatenate([w_o[:, h * 64:(h + 1) * 64] for h in heads], 1).T
        ).astype(bf))
        in_maps.append({
            "hsT": hs2, "wqkT": wqkT, "wvT": wvT, "woT": woT,
            "cos2T": cos2T, "ssin2T": ssin2T, "maskD": maskD, "pmat": pmat,
        })
    return in_maps


def unswizzle_out(o2):
    # [128, qi*half*oi*512] -> outT [1024, 2048]
    a = o2.reshape(128, NQC, 2, 4, QC)
    return np.ascontiguousarray(
        a.transpose(2, 3, 0, 1, 4).reshape(HID, S))


def unshard(outTs):
    out = np.zeros((B, S, HID), np.float32)
    for c, o2 in enumerate(outTs):
        out[c // 4] += unswizzle_out(o2).T.astype(np.float32)
    return out


# ---------- standalone kernel entry ----------

from concourse.bass_utils import run_bass_kernel_spmd

_CACHED_NC = None


def get_program():
    global _CACHED_NC
    if _CACHED_NC is None:
        _CACHED_NC = build_program()
    return _CACHED_NC


def run(inputs, trace=False):
    nc = get_program()
    in_maps = make_core_inputs(**inputs)
    res = run_bass_kernel_spmd(nc, in_maps, core_ids=list(range(8)), trace=trace)
    out = np.zeros((B, S, HID), np.float32)
    for c, r in enumerate(res.results):
        out[c // 4] += unswizzle_out(r["outT"]).T.astype(np.float32)
    return out, res


def kernel(**inputs):
    out, _ = run(inputs, trace=False)
    return out



# revision 43
# speedup vs baseline: 1.3603x; 1.0355x over previous
"""Sharded causal-attention kernel for 8 trn2 NeuronCores.

DP over batch (2) x TP over head groups (4 heads/core). Each core: qkv projection
(its heads) + RoPE + causal SDPA (scores kept transposed; softmax denominator via a
ones-column in the PV matmul) + its 256-row slice of the o_proj contraction, returning
a transposed partial [HID, S]; the host sums 4 partials per batch. bf16 matmuls,
fp32 PSUM accumulation.

v4 = v2 schedule skeleton with the ACT engine reserved for exp (its 72us pole):
every PSUM->SBUF copy (RoPE x, v, numerators, denominator rows, o_proj out)
moved to DVE with denominator rows copied first so the recip/broadcast chain
starts earliest; RoPE t1 mul and final add moved to GpSimd (SBUF-only ops,
~780ns each, keeps DVE under the PE pole); rb-major wqk layout so the first
projection row-block gates on one 256KB transfer; hs chunks 1-3 as single
8-piece transfers on the gpsimd queue.
"""

import sys

sys.path.insert(0, "/opt/trn_rl_repo")

from contextlib import ExitStack

import numpy as np
import ml_dtypes

import concourse.bass as bass
import concourse.mybir as mybir
import concourse.tile as tile
from concourse import bacc

FP = mybir.dt.float32
BF = mybir.dt.bfloat16
EXP = mybir.ActivationFunctionType.Exp

B, S, HID = 2, 2048, 1024
H, D = 16, 64
QC = 512
KT = 128
NQC = S // QC
NKT = S // KT
KHID = HID // 128


def build_program(debug_outputs=False):
    nc = bacc.Bacc("TRN2", target_bir_lowering=False, debug=False, num_devices=8, num_swdge_queues=4)

    hsT = nc.dram_tensor("hsT", [128, NQC * KHID * QC], BF, kind="ExternalInput").ap()
    wqkT = nc.dram_tensor("wqkT", [128, KHID * 512], BF, kind="ExternalInput").ap()
    wvT = nc.dram_tensor("wvT", [128, KHID * 256], BF, kind="ExternalInput").ap()
    woT = nc.dram_tensor("woT", [128, 2 * HID], BF, kind="ExternalInput").ap()
    cos2T = nc.dram_tensor("cos2T", [64, S], BF, kind="ExternalInput").ap()
    ssin2T = nc.dram_tensor("ssin2T", [64, S], BF, kind="ExternalInput").ap()
    maskD = nc.dram_tensor("maskD", [128, 512], BF, kind="ExternalInput").ap()
    pmat = nc.dram_tensor("pmat", [128, 128], BF, kind="ExternalInput").ap()
    outT = nc.dram_tensor("outT", [128, NQC * 8 * QC], BF, kind="ExternalOutput").ap()
    dbg = None
    if debug_outputs:
        dbg = {
            "dbg_qk": nc.dram_tensor("dbg_qk", [512, S], BF, kind="ExternalOutput").ap(),
            "dbg_v": nc.dram_tensor("dbg_v", [128, NKT * 4 * 65], BF, kind="ExternalOutput").ap(),
            "dbg_att": nc.dram_tensor("dbg_att", [256, S], BF, kind="ExternalOutput").ap(),
        }

    with tile.TileContext(nc) as tc:
        build_tile_program(tc, hsT, wqkT, wvT, woT, cos2T, ssin2T, maskD, pmat, outT, dbg)
    nc.compile()
    return nc


def build_tile_program(tc, hsT, wqkT, wvT, woT, cos2T, ssin2T, maskD, pmat, outT, dbg=None):
    nc = tc.nc
    with ExitStack() as ctx:
        const = ctx.enter_context(tc.tile_pool(name="const", bufs=1))
        persist = ctx.enter_context(tc.tile_pool(name="persist", bufs=1))
        work = ctx.enter_context(tc.tile_pool(name="work", bufs=5))
        posbp = ctx.enter_context(tc.tile_pool(name="posbp", bufs=12))
        expp = ctx.enter_context(tc.tile_pool(name="expp", bufs=8))
        small = ctx.enter_context(tc.tile_pool(name="small", bufs=4))
        ps_main = ctx.enter_context(tc.tile_pool(name="ps_main", bufs=4, space="PSUM"))
        ps_sc = ctx.enter_context(tc.tile_pool(name="ps_sc", bufs=2, space="PSUM"))

        # ---- critical first wave: exactly what the first proj matmuls gate on,
        # in consumption order (rb-major wqk so proj rb0 gates on 256KB),
        # spread across the three DMA queues ----
        wqk_sb = const.tile([128, 4, KHID, 128], BF, name="wqk_sb")
        hs_sb = const.tile([128, NQC, KHID, QC], BF, name="hs_sb")
        cos_sb = const.tile([128, S], BF, name="cos_sb")
        ssin_sb = const.tile([128, S], BF, name="ssin_sb")
        pmat_sb = const.tile([128, 128], BF, name="pmat_sb")
        tri_sb = const.tile([128, 4, 128], BF, name="tri_sb")
        wv_sb = const.tile([128, KHID, 256], BF, name="wv_sb")
        wo_sb = const.tile([128, 2, HID], BF, name="wo_sb")

        def wq_rb(rb, eng):
            eng.dma_start(
                wqk_sb[:, rb],
                wqkT[:, rb * 1024:(rb + 1) * 1024].rearrange("p (k m) -> p k m", k=KHID),
            )

        def hs_piece(t, klo, khi, eng):
            eng.dma_start(
                hs_sb[:, t, klo:khi, :],
                hsT[:, (t * KHID + klo) * QC:(t * KHID + khi) * QC].rearrange(
                    "p (k s) -> p k s", k=khi - klo),
            )

        wq_rb(0, nc.sync)
        hs_piece(0, 0, 2, nc.gpsimd)
        nc.scalar.dma_start(cos_sb[0:64, 0:QC], cos2T[:, 0:QC])
        hs_piece(0, 2, 4, nc.gpsimd)
        nc.scalar.dma_start(ssin_sb[0:64, 0:QC], ssin2T[:, 0:QC])
        nc.scalar.dma_start(pmat_sb[:], pmat[:])
        wq_rb(2, nc.scalar)
        hs_piece(0, 4, 6, nc.sync)
        wq_rb(1, nc.sync)
        hs_piece(0, 6, 8, nc.gpsimd)
        wq_rb(3, nc.sync)
        # remaining hs chunks ride the gpsimd queue behind the chunk-0 pieces
        hs_piece(1, 0, 8, nc.gpsimd)
        hs_piece(2, 0, 8, nc.gpsimd)
        hs_piece(3, 0, 8, nc.gpsimd)
        nc.vector.tensor_copy(cos_sb[64:128, 0:QC], cos_sb[0:64, 0:QC])
        nc.vector.tensor_copy(ssin_sb[64:128, 0:QC], ssin_sb[0:64, 0:QC])

        qkT = persist.tile([128, 4, S], BF, name="qkT")
        l_tiles = persist.tile([64, 8, QC], FP, name="l_tiles")
        v_sb = persist.tile([128, NKT, 4 * 65], BF, name="v_sb2")
        att_sb = persist.tile([128, 2, S], BF, name="att_sb2")
        nc.vector.memset(
            v_sb.rearrange("p t (h c) -> p t h c", c=65)[:, :, :, 64:65], 1.0
        )

        def proj_chunk(rb, t):
            csl = slice(t * QC, (t + 1) * QC)
            ps = ps_main.tile([128, QC], FP, name="ps_qk", tag="ps")
            for kk in range(KHID):
                nc.tensor.matmul(
                    ps[:],
                    wqk_sb[:, rb, kk, :],
                    hs_sb[:, t, kk, :],
                    start=(kk == 0),
                    stop=(kk == KHID - 1),
                )
            x = work.tile([128, QC], BF, name="x_rope", tag="xrope")
            half = QC // 2
            nc.scalar.copy(x[:, 0:half], ps[:, 0:half])
            nc.vector.tensor_copy(x[:, half:QC], ps[:, half:QC])
            # signed rotate-half on the PE: xs = P @ x (P carries the +-1),
            # split by column half so each starts as soon as its copy lands
            xs_ps = ps_main.tile([128, QC], FP, name="xs_ps", tag="ps")
            nc.tensor.matmul(xs_ps[:, 0:half], pmat_sb[:], x[:, 0:half], start=True, stop=True)
            nc.tensor.matmul(xs_ps[:, half:QC], pmat_sb[:], x[:, half:QC], start=True, stop=True)
            t1 = work.tile([128, QC], BF, name="t1_rope", tag="t1rope")
            t2 = work.tile([128, QC], BF, name="t2_rope", tag="t2rope")
            nc.vector.tensor_mul(t1[:], x[:], cos_sb[:, csl])
            nc.vector.tensor_mul(t2[:], xs_ps[:], ssin_sb[:, csl])
            nc.vector.tensor_add(qkT[:, rb, csl], t1[:], t2[:])

        def v_proj(tt):
            psv = ps_main.tile([128, 256], FP, name="ps_v", tag="ps")
            for kk in range(KHID):
                nc.tensor.matmul(
                    psv[:],
                    hs_sb[:, tt // 4, kk, (tt % 4) * 128:(tt % 4 + 1) * 128],
                    wv_sb[:, kk, :],
                    start=(kk == 0),
                    stop=(kk == KHID - 1),
                )
            nc.vector.tensor_copy(
                v_sb[:, tt, :].rearrange("p (h c) -> p h c", c=65)[:, :, 0:64],
                psv[:].rearrange("p (h c) -> p h c", c=64),
            )

        def attention_unit(qi, l0, l1):
            """Fused scores^T -> exp -> PV for all four heads at q-chunk qi.

            Both head-pairs interleave per k-tile so each exp has ~2x the PE
            cover, and the PV batch trails one k-tile behind. The four PV
            accumulators occupy all four ps_main buffers for the unit.
            Denominators land in l0/l1 rows 0 and 32."""
            qsl = slice(qi * QC, (qi + 1) * QC)
            nki = 4 * qi + 4
            po = [ps_main.tile([65, QC], FP, name=f"po{h}", tag="ps") for h in range(4)]

            def emit_pv(ki, e_, lo):
                for h in range(4):
                    nc.tensor.matmul(
                        po[h][:, lo:QC], v_sb[:, ki, h * 65:(h + 1) * 65],
                        e_[:, h, lo:QC],
                        start=(ki == 0), stop=(ki == nki - 1),
                    )

            pending = []
            for ki in range(nki):
                ksl = slice(ki * KT, (ki + 1) * KT)
                j = ki - 4 * qi
                lo = 0 if j < 0 else 128 * j  # first live q column in this chunk
                e = expp.tile([128, 4, QC], BF, name="e", tag="exp", bufs=4)
                for pair in range(2):
                    psc = ps_sc.tile([128, 2, QC], FP, name="psc", tag="sc")
                    nc.tensor.matmul(
                        psc[:, 0, lo:QC], qkT[0:64, 2 + pair, ksl],
                        qkT[0:64, pair, qi * QC + lo:(qi + 1) * QC],
                        start=True, stop=True,
                    )
                    nc.tensor.matmul(
                        psc[:, 1, lo:QC], qkT[64:128, 2 + pair, ksl],
                        qkT[64:128, pair, qi * QC + lo:(qi + 1) * QC],
                        start=True, stop=True,
                    )
                    nc.scalar.activation(
                        e[:, 2 * pair:2 * pair + 2, lo:QC], psc[:, :, lo:QC],
                        EXP, scale=0.125,
                    )
                if j >= 0:
                    nc.vector.tensor_mul(
                        e[:, :, lo:lo + 128], e[:, :, lo:lo + 128], tri_sb[:]
                    )
                if pending:
                    emit_pv(*pending.pop(0))
                pending.append((ki, e, lo))
            for p_ in pending:
                emit_pv(*p_)
            # epilogue part 1 (all DVE so exps never queue behind copies):
            # just the denominator rows, so the recip/broadcast chain starts
            # earliest; numerator copies follow in unit_numerators()
            nc.vector.tensor_copy(l0[0:1, :], po[0][64:65, :])
            nc.vector.tensor_copy(l0[32:33, :], po[1][64:65, :])
            nc.vector.tensor_copy(l1[0:1, :], po[2][64:65, :])
            nc.vector.tensor_copy(l1[32:33, :], po[3][64:65, :])
            return po

        def unit_numerators(po):
            po_sb = []
            for h in range(4):
                t_ = posbp.tile([64, QC], BF, name=f"po_sb{h}", tag="posb")
                nc.vector.tensor_copy(t_[:], po[h][0:64, :])
                po_sb.append(t_)
            return (po_sb[0], po_sb[1]), (po_sb[2], po_sb[3])

        def division_pre(l_pair):
            """1/l for both heads of a pair, broadcast to 64 partitions.

            Only depends on the denominator rows, so the gpsimd broadcasts
            overlap the next attention unit / o_proj on the PE."""
            rl = small.tile([64, QC], FP, name="rl", tag="rl")
            nc.vector.reciprocal_approx_fast(out=rl[:], in_=l_pair[:])
            rb0_ = small.tile([64, QC], FP, name="rb0_", tag="rbb", bufs=6)
            nc.gpsimd.partition_broadcast(rb0_[:], rl[0:1, :])
            rlrow = small.tile([1, QC], FP, name="rlrow", tag="rlrow", bufs=4)
            nc.vector.tensor_copy(rlrow[:], rl[32:33, :])
            rb1_ = small.tile([64, QC], FP, name="rb1_", tag="rbb", bufs=6)
            nc.gpsimd.partition_broadcast(rb1_[:], rlrow[:])
            return rb0_, rb1_

        def division_post(pair, qi, rb, po_sb0, po_sb1):
            qsl = slice(qi * QC, (qi + 1) * QC)
            rb0_, rb1_ = rb
            nc.vector.tensor_mul(att_sb[0:64, pair, qsl], po_sb0[0:64, :], rb0_[:])
            nc.vector.tensor_mul(att_sb[64:128, pair, qsl], po_sb1[0:64, :], rb1_[:])

        def oproj(qi, last=False, tailish=False):
            qsl = slice(qi * QC, (qi + 1) * QC)
            out_engs = (nc.gpsimd, nc.sync, nc.scalar, nc.gpsimd)
            if last:
                for half in range(2):
                    ow = work.tile([128, 4, QC], BF, name="ow", tag="ow")
                    for oi in range(4):
                        ot = half * 4 + oi
                        pw = ps_main.tile([128, QC], FP, name="pw", tag="ps")
                        for p in range(2):
                            nc.tensor.matmul(
                                pw[:],
                                wo_sb[:, p, ot * 128:(ot + 1) * 128],
                                att_sb[:, p, qsl],
                                start=(p == 0),
                                stop=(p == 1),
                            )
                        nc.scalar.copy(ow[:, oi, 0:QC // 2], pw[:, 0:QC // 2])
                        nc.vector.tensor_copy(ow[:, oi, QC // 2:QC], pw[:, QC // 2:QC])
                        off = (qi * 8 + half * 4 + oi) * QC
                        out_engs[oi].dma_start(outT[:, off:off + QC], ow[:, oi, :])
                return
            for half in range(2):
                ow = work.tile([128, 4, QC], BF, name="ow", tag="ow")
                for oi in range(4):
                    ot = half * 4 + oi
                    pw = ps_main.tile([128, QC], FP, name="pw", tag="ps")
                    for p in range(2):
                        nc.tensor.matmul(
                            pw[:],
                            wo_sb[:, p, ot * 128:(ot + 1) * 128],
                            att_sb[:, p, qsl],
                            start=(p == 0),
                            stop=(p == 1),
                        )
                    if (oi % 2 == 1) if tailish else (ot % 2 == 1):
                        nc.scalar.copy(ow[:, oi, :], pw[:])
                    else:
                        nc.vector.tensor_copy(ow[:, oi, :], pw[:])
                off = (qi * 2 + half) * 4 * QC
                (nc.gpsimd if half == 0 else nc.sync).dma_start(
                    outT[:, off:off + 4 * QC].rearrange("p (o s) -> p o s", o=4),
                    ow[:],
                )

        # emission: pair0 projections up front with the deferred weight loads
        # slotted behind compute-gated ops so they don't steal SDMA bandwidth
        # from the chunk-0 / wqk gates; then pair1 projections, v, attention
        # and (one chunk behind) o_proj interleaved per q chunk.
        for t in range(NQC):
            if t == 1:
                nc.vector.tensor_copy(cos_sb[64:128, QC:S], cos_sb[0:64, QC:S])
                nc.vector.tensor_copy(ssin_sb[64:128, QC:S], ssin_sb[0:64, QC:S])
            proj_chunk(0, t)
            if t == 0:
                nc.scalar.dma_start(cos_sb[0:64, QC:S], cos2T[:, QC:S])
                nc.scalar.dma_start(ssin_sb[0:64, QC:S], ssin2T[:, QC:S])
                nc.scalar.dma_start(tri_sb[:], maskD.rearrange("p (r c) -> p r c", r=4))
            proj_chunk(2, t)
            if t == 0:
                for h in range(2):
                    nc.scalar.dma_start(
                        wv_sb[:, 4 * h:4 * h + 4, :],
                        wvT[:, h * 1024:(h + 1) * 1024].rearrange("p (k m) -> p k m", k=4),
                    )
            if t == 1:
                for h in range(2):
                    nc.scalar.dma_start(wo_sb[:, h, :], woT[:, h * HID:(h + 1) * HID])
        nc.vector.memset(l_tiles[:], 1.0)
        qi_order = [1, 0, 2, 3]
        loaded = 0
        prev = None
        for qi in qi_order:
            while loaded <= min(qi + 1, NQC - 1):
                proj_chunk(1, loaded)
                proj_chunk(3, loaded)
                for tt in range(4 * loaded, 4 * loaded + 4):
                    v_proj(tt)
                loaded += 1
            l0 = l_tiles[:, 2 * qi, :]
            l1 = l_tiles[:, 2 * qi + 1, :]
            po = attention_unit(qi, l0, l1)
            rlb0 = division_pre(l0)
            rlb1 = division_pre(l1)
            pa, pb = unit_numerators(po)
            if prev is not None:
                oproj(prev, tailish=(qi == qi_order[-1]))
            division_post(0, qi, rlb0, *pa)
            division_post(1, qi, rlb1, *pb)
            prev = qi
        oproj(prev, last=True)

        if dbg is not None:
            for rb in range(4):
                nc.sync.dma_start(dbg["dbg_qk"][rb * 128:(rb + 1) * 128, :], qkT[:, rb, :])
            nc.sync.dma_start(dbg["dbg_v"][:], v_sb.rearrange("p t c -> p (t c)"))
            for p in range(2):
                nc.sync.dma_start(dbg["dbg_att"][p * 128:(p + 1) * 128, :], att_sb[:, p, :])


# ---------- host-side shard preparation ----------

def make_core_inputs(hidden_states, cos, sin, w_qkv, w_o):
    """Returns list of 8 in_maps (numpy, bf16 where needed)."""
    bf = ml_dtypes.bfloat16
    hs = np.asarray(hidden_states, np.float32)
    cos = np.asarray(cos, np.float32)
    sin = np.asarray(sin, np.float32)
    w_qkv = np.asarray(w_qkv, np.float32)
    w_o = np.asarray(w_o, np.float32)

    cosT = cos.T
    sinT = sin.T
    cos2T = np.ascontiguousarray(cosT).astype(bf)
    ssin2T = np.ascontiguousarray(sinT).astype(bf)
    # signed rotate-half permutation: out[m] = sign(m) * x[partner(m)]
    # lhsT layout: pmat[k, m] = sign(m) at k = partner(m)
    pmat = np.zeros((128, 128), np.float32)
    for m in range(128):
        d = m % 64
        base = m - d
        if d < 32:
            pmat[base + d + 32, m] = -1.0
        else:
            pmat[base + d - 32, m] = 1.0
    pmat = pmat.astype(bf)

    kp = np.arange(128)[:, None]
    cc = np.arange(128)[None, :]
    tri = (kp <= cc).astype(bf)
    maskD = np.concatenate([tri, tri, tri, tri], axis=1)

    def swz(a):
        # [K*128, M] -> [128, K*M] partition-major (matches SBUF tiles)
        k = a.shape[0] // 128
        return np.ascontiguousarray(
            a.reshape(k, 128, a.shape[1]).transpose(1, 0, 2).reshape(128, -1))

    in_maps = []
    for c in range(8):
        b, g = divmod(c, 4)
        heads = range(4 * g, 4 * g + 4)
        hsT = np.ascontiguousarray(hs[b].T).astype(bf)  # [HID, S]
        # chunk-major swizzle: [128, t, kk, s']
        hs2 = np.ascontiguousarray(
            hsT.reshape(8, 128, 4, 512).transpose(1, 2, 0, 3).reshape(128, -1))
        wq = np.concatenate([w_qkv[h * 64:(h + 1) * 64] for h in heads], 0)
        wk = np.concatenate([w_qkv[HID + h * 64:HID + (h + 1) * 64] for h in heads], 0)
        wv = np.concatenate([w_qkv[2 * HID + h * 64:2 * HID + (h + 1) * 64] for h in heads], 0)
        # rb-major wqk: [128, rb, kk, m] so proj of one 128-row block gates
        # on a single 256KB transfer
        wqk_cat = np.concatenate([wq, wk], 0).astype(np.float32)  # [512, 1024]
        arr = np.ascontiguousarray(wqk_cat.T)  # [1024, 512] = [kk*128+p, rb*128+m]
        wqkT = np.ascontiguousarray(
            arr.reshape(KHID, 128, 4, 128).transpose(1, 2, 0, 3).reshape(128, -1)
        ).astype(bf)
        wvT = swz(np.ascontiguousarray(wv.T).astype(bf))
        woT = swz(np.ascontiguousarray(
            np.concatenate([w_o[:, h * 64:(h + 1) * 64] for h in heads], 1).T
        ).astype(bf))
        in_maps.append({
            "hsT": hs2, "wqkT": wqkT, "wvT": wvT, "woT": woT,
            "cos2T": cos2T, "ssin2T": ssin2T, "maskD": maskD, "pmat": pmat,
        })
    return in_maps


def unswizzle_out(o2):
    # [128, qi*half*oi*512] -> outT [1024, 2048]
    a = o2.reshape(128, NQC, 2, 4, QC)
    return np.ascontiguousarray(
        a.transpose(2, 3, 0, 1, 4).reshape(HID, S))


def unshard(outTs):
    out = np.zeros((B, S, HID), np.float32)
    for c, o2 in enumerate(outTs):
        out[c // 4] += unswizzle_out(o2).T.astype(np.float32)
    return out


# ---------- standalone kernel entry ----------

from concourse.bass_utils import run_bass_kernel_spmd

_CACHED_NC = None


def get_program():
    global _CACHED_NC
    if _CACHED_NC is None:
        _CACHED_NC = build_program()
    return _CACHED_NC


def run(inputs, trace=False):
    nc = get_program()
    in_maps = make_core_inputs(**inputs)
    res = run_bass_kernel_spmd(nc, in_maps, core_ids=list(range(8)), trace=trace)
    out = np.zeros((B, S, HID), np.float32)
    for c, r in enumerate(res.results):
        out[c // 4] += unswizzle_out(r["outT"]).T.astype(np.float32)
    return out, res


def kernel(**inputs):
    out, _ = run(inputs, trace=False)
    return out

